# revision 1
# baseline (speedup 1.0000x reference)
"""Bidirectional column-chained GRU (vertical BiGRU over image columns) on 8 Trainium2 cores.

Topology: cores 0-3 run the forward GRU chain (batch quarters), cores 4-7 the
backward chain (rows pre-reversed on host). Each core runs the full C*S=16384
sequential GRU steps for its 8 batch rows in feature-major layout (128
partitions = hidden dim, free dim = batch).

Math restructuring (validated vs reference in numpy):
  state hp1 = h + 1  (so n-path affine folds shrink the serial chain)
  tanh(x) = 2*sigmoid(2x) - 1  (single ACT table: sigmoid set, no switches)
  Per column c, for each gate g the rank-1 input contribution
  A_g,t = Wih_g*x_t + const_g is preloaded into PSUM with K=2 matmuls
  (const corrected by -Whh_g@1 for the hp1 shift); the recurrent matmul
  Whh_g @ hp1 then accumulates per step into PSUM slice t, so the full
  pre-activation is read directly from PSUM by ACT/DVE.
  Per step:
    r  = sigmoid(ps_r[t])                 ACT (PSUM src)
    u  = sigmoid(-ps_z[t])  (= 1-z)       ACT
    q  = r * ps_n[t]                      DVE
    w  = q + A_n[t]                       DVE
    v  = sigmoid(2w)                      ACT
    e1 = u * hp1; f = hp1 - e1            DVE
    e2 = 2*u*v                            DVE (scalar_tensor_tensor)
    hp1' = f + e2                         DVE
  Matmuls issue in r, n, z order: ps_r gates the chain head and ps_n is
  needed mid-chain by q, so the three serial fp32 weight loads put ps_n
  ahead of its deadline; ps_z (read by the slack-rich u path) goes last.
  The second half-column's preload matmuls are emitted after step 8 of the
  first half so they execute in PE idle time instead of serializing at the
  half boundary.
  Final per-column features h = hp1 - 1 are collected; the output head
  (fc + relu + softmax) runs on-device with a pairwise AllReduce between the
  fwd/bwd core of each batch group. exp(relu(x)) == max(1, exp(x)).
"""

import numpy as np

import concourse.bass as bass
import concourse.bacc as bacc
import concourse.mybir as mybir
import concourse.tile as tile
from concourse.bass_utils import run_bass_kernel_spmd

B, S, C, H, O = 32, 128, 128, 128, 64
NCORES = 8
BL = B // 4          # batch rows per core (4 groups x 2 directions)
SB = S * BL          # rhs columns per image column
HS = SB // 2         # half-column psum width (one bank)
NSTEP = S // 2       # steps per half
f32 = mybir.dt.float32
FP = mybir.EngineType


def _emit(nc: bacc.Bacc, n_cols: int = C, loop_cols: int | None = None, skip_collective: bool = False, zero_hall: bool = False, mm_dt=mybir.dt.float32, split_r: bool = False, sph: int = NSTEP, zs: bool = True):
    AF = mybir.ActivationFunctionType
    OPM = mybir.AluOpType.mult

    xaug_d = nc.dram_tensor("xaug", [n_cols * 2, SB], f32, kind="ExternalInput").ap()
    hp10_d = nc.dram_tensor("hp10", [H, BL], f32, kind="ExternalInput").ap()
    whhrT_d = nc.dram_tensor("whhrT", [H, H], f32, kind="ExternalInput").ap()
    whhzT_d = nc.dram_tensor("whhzT", [H, H], f32, kind="ExternalInput").ap()
    whhnT_d = nc.dram_tensor("whhnT", [H, H], f32, kind="ExternalInput").ap()
    lcat_d = nc.dram_tensor("lcat", [2, 4 * H], f32, kind="ExternalInput").ap()
    wfcT_d = nc.dram_tensor("wfcT", [H, O], f32, kind="ExternalInput").ap()
    bias_d = nc.dram_tensor("bias_bc", [H, 8 * O], f32, kind="ExternalInput").ap()
    out_d = nc.dram_tensor("out", [C * BL, O], f32, kind="ExternalOutput").ap()

    with tile.TileContext(nc) as tc:
        with tc.tile_pool(name="const", bufs=1) as cp:
            whhrT = cp.tile([H, H], f32)
            whhzT = cp.tile([H, H], f32)
            whhnT = cp.tile([H, H], f32)
            lcat = cp.tile([2, 4 * H], f32)
            wfcT = cp.tile([H, O], f32)
            biasb = cp.tile([H, 8 * O], f32)
            hp1 = cp.tile([H, BL], f32)
            hall = cp.tile([H, C * BL], f32)
            r = cp.tile([H, BL], f32)
            u = cp.tile([H, BL], f32)
            q = cp.tile([H, BL], f32)
            w = cp.tile([H, BL], f32)
            v = cp.tile([H, BL], f32)
            e1 = cp.tile([H, BL], f32)
            if zs:
                fe2 = cp.tile([H, 2 * BL], f32)
                fp_, e2 = fe2[:, 0:BL], fe2[:, BL : 2 * BL]
            else:
                fp_ = cp.tile([H, BL], f32)
                e2 = cp.tile([H, BL], f32)

            if zero_hall:
                nc.gpsimd.memset(hall[:], 0.0)
            nc.sync.dma_start(whhrT[:], whhrT_d)
            nc.sync.dma_start(whhzT[:], whhzT_d)
            nc.sync.dma_start(whhnT[:], whhnT_d)
            nc.sync.dma_start(lcat[:], lcat_d)
            nc.sync.dma_start(wfcT[:], wfcT_d)
            nc.sync.dma_start(biasb[:], bias_d)
            nc.sync.dma_start(hp1[:], hp10_d)
            if split_r or zs:
                nc.sync.dma_start(fp_[:], hp10_d)
                nc.vector.memzero(e2[:])

            with (
                tc.tile_pool(name="col", bufs=2) as colp,
                tc.tile_pool(name="ps", bufs=2, space="PSUM") as psp,
                tc.For_i(
                    0, n_cols if loop_cols is None else loop_cols, 1,
                    hint_engines=(FP.PE, FP.Activation, FP.DVE),
                ) as cv,
            ):
                xa = colp.tile([2, SB], f32, tag="xa")
                nc.sync.dma_start(xa[:], xaug_d[bass.ds(cv * 2, 2), :])

                def preload(half):
                    ps_r = psp.tile([H, HS], f32, tag="ps_r", name=f"ps_r{half}")
                    ps_z = psp.tile([H, HS], f32, tag="ps_z", name=f"ps_z{half}")
                    ps_n = psp.tile([H, HS], f32, tag="ps_n", name=f"ps_n{half}")
                    ps_t = psp.tile([H, HS], f32, tag="ps_t", name=f"ps_t{half}")
                    a_n = colp.tile([H, HS], f32, tag="a_n", name=f"a_n{half}")
                    xh = xa[:, half * HS : (half + 1) * HS].bitcast(mm_dt)
                    lcv = lcat[:].bitcast(mm_dt)
                    nc.tensor.matmul(ps_r[:], lcv[:, 0:H], xh, start=True, stop=True)
                    nc.tensor.matmul(ps_z[:], lcv[:, H : 2 * H], xh, start=True, stop=True)
                    nc.tensor.matmul(ps_n[:], lcv[:, 2 * H : 3 * H], xh, start=True, stop=True)
                    nc.tensor.matmul(ps_t[:], lcv[:, 3 * H : 4 * H], xh, start=True, stop=True)
                    nc.scalar.copy(a_n[:], ps_t[:])
                    return ps_r, ps_z, ps_n, a_n

                def steps(ph, lo, hi):
                    ps_r, ps_z, ps_n, a_n = ph
                    for t in range(lo, hi):
                        sl = slice(t * BL, (t + 1) * BL)
                        wrT = whhrT[:].bitcast(mm_dt)
                        wzT = whhzT[:].bitcast(mm_dt)
                        wnT = whhnT[:].bitcast(mm_dt)
                        if zs:
                            hp1v = fe2[:].bitcast(mm_dt).rearrange(
                                "p (a o) -> p a o", a=2
                            )
                        else:
                            hp1v = hp1[:].bitcast(mm_dt)
                        if zs:
                            outs = [
                                bass.broadcast_tensor_aps(
                                    ps[:, sl].rearrange("p (a o) -> p a o", a=1),
                                    hp1v,
                                )[0]
                                for ps in (ps_r, ps_n, ps_z)
                            ]
                        else:
                            outs = [ps_r[:, sl], ps_n[:, sl], ps_z[:, sl]]
                        for o_, w_ in zip(outs, (wrT, wnT, wzT)):
                            nc.tensor.matmul(
                                o_, w_, hp1v, start=False, stop=True,
                                skip_group_check=True,
                            )
                        nc.scalar.activation(r[:], ps_r[:, sl], AF.Sigmoid)
                        nc.scalar.activation(u[:], ps_z[:, sl], AF.Sigmoid, scale=-1.0)
                        nc.vector.tensor_mul(q[:], r[:], ps_n[:, sl])
                        nc.vector.tensor_add(w[:], q[:], a_n[:, sl])
                        nc.scalar.activation(v[:], w[:], AF.Sigmoid, scale=2.0)
                        nc.vector.tensor_mul(e1[:], u[:], hp1[:])
                        nc.vector.tensor_sub(fp_[:], hp1[:], e1[:])
                        nc.vector.scalar_tensor_tensor(
                            e2[:], u[:], 2.0, v[:], op0=OPM, op1=OPM
                        )
                        nc.vector.tensor_add(hp1[:], fp_[:], e2[:])

                ph0 = preload(0)
                steps(ph0, 0, min(8, sph))
                ph1 = preload(1)
                steps(ph0, 8, max(8, sph))
                steps(ph1, 0, sph)
                nc.vector.tensor_scalar_add(
                    hall[:, bass.ts(cv, BL)], hp1[:], -1.0
                )

            # output head: partial logits -> allreduce(fwd,bwd) -> softmax(relu(.))
            with (
                tc.tile_pool(name="fc", bufs=1) as fcp,
                tc.tile_pool(name="psfc", bufs=1, space="PSUM") as psfc,
                tc.tile_pool(name="dramp", bufs=1, space="DRAM") as dp,
            ):
                lps = psfc.tile([128, 8 * O], f32)
                for k in range(8):
                    nc.tensor.matmul(
                        lps[:, k * O : (k + 1) * O],
                        hall[:, k * 128 : (k + 1) * 128],
                        wfcT[:],
                        start=True,
                        stop=True,
                    )
                lsb = fcp.tile([128, 8 * O], f32)
                nc.scalar.copy(lsb[:], lps[:])
                lloc = dp.tile([C * BL, O], f32)
                lred = dp.tile([C * BL, O], f32)
                nc.sync.dma_start(
                    lloc.rearrange("(k p) o -> p k o", p=128),
                    lsb[:].rearrange("p (k o) -> p k o", k=8),
                )
                if skip_collective:
                    nc.sync.dma_start(lred[:], lloc[:])
                else:
                    nc.gpsimd.collective_compute(
                        "AllReduce",
                        mybir.AluOpType.add,
                        replica_groups=[[0, 4], [1, 5], [2, 6], [3, 7]],
                        ins=[lloc.opt()],
                        outs=[lred.opt()],
                    )
                lsum = fcp.tile([128, 8 * O], f32)
                nc.sync.dma_start(
                    lsum[:].rearrange("p (k o) -> p k o", k=8),
                    lred.rearrange("(k p) o -> p k o", p=128),
                )
                lbi = fcp.tile([128, 8 * O], f32)
                nc.vector.tensor_add(lbi[:], lsum[:], biasb[:])
                ex = fcp.tile([128, 8 * O], f32)
                nc.scalar.activation(ex[:], lbi[:], AF.Exp)
                # exp(relu(x)) == max(1, exp(x))
                nc.vector.tensor_scalar_max(ex[:], ex[:], 1.0)
                sums = fcp.tile([128, 8], f32)
                nc.vector.tensor_reduce(
                    sums[:],
                    ex[:].rearrange("p (k o) -> p k o", k=8),
                    axis=mybir.AxisListType.X,
                    op=mybir.AluOpType.add,
                )
                rs = fcp.tile([128, 8], f32)
                nc.vector.reciprocal(rs[:], sums[:])
                osb = fcp.tile([128, 8 * O], f32)
                for k in range(8):
                    nc.vector.tensor_scalar_mul(
                        osb[:, k * O : (k + 1) * O],
                        ex[:, k * O : (k + 1) * O],
                        rs[:, k : k + 1],
                    )
                nc.sync.dma_start(
                    out_d.rearrange("(k p) o -> p k o", p=128),
                    osb[:].rearrange("p (k o) -> p k o", k=8),
                )


_CACHE = {}


def _build():
    if "nc" not in _CACHE:
        nc = bacc.Bacc("TRN2", target_bir_lowering=False, debug=False, num_devices=NCORES)
        _emit(nc)
        nc.compile()
        _CACHE["nc"] = nc
    return _CACHE["nc"]


def _core_inputs(inputs, d, g):
    """Host-side prep for core (direction d, batch group g)."""
    bsl = slice(g * BL, (g + 1) * BL)
    x = inputs["x"][bsl]
    if d == 1:
        x = x[:, ::-1, :]
    xT = np.ascontiguousarray(np.transpose(x, (2, 1, 0)))  # (C, S, BL)
    xcols = xT.reshape(C, SB)
    xaug = np.empty((C * 2, SB), np.float32)
    xaug[0::2] = xcols
    xaug[1::2] = 1.0
    sfx = "f" if d == 0 else "b"
    Wih = inputs[f"Wih_{sfx}"][:, 0]
    Whh = inputs[f"Whh_{sfx}"]
    bih = inputs[f"bih_{sfx}"]
    bhh = inputs[f"bhh_{sfx}"]
    Wr, Wz, Wn = Whh[:H], Whh[H : 2 * H], Whh[2 * H :]
    lcat = np.zeros((2, 4 * H), np.float32)
    lcat[0, 0:H] = Wih[:H]
    lcat[1, 0:H] = bih[:H] + bhh[:H] - Wr.sum(1)
    lcat[0, H : 2 * H] = Wih[H : 2 * H]
    lcat[1, H : 2 * H] = bih[H : 2 * H] + bhh[H : 2 * H] - Wz.sum(1)
    lcat[1, 2 * H : 3 * H] = bhh[2 * H :] - Wn.sum(1)
    lcat[0, 3 * H : 4 * H] = Wih[2 * H :]
    lcat[1, 3 * H : 4 * H] = bih[2 * H :]
    wfc_half = inputs["W_fc"][:, :H] if d == 0 else inputs["W_fc"][:, H:]
    bias_bc = np.tile(inputs["b_fc"], (H, 8)).astype(np.float32)
    return {
        "xaug": xaug,
        "hp10": np.ascontiguousarray((inputs["h_prev"][d, bsl] + 1.0).T).astype(
            np.float32
        ),
        "whhrT": np.ascontiguousarray(Wr.T).astype(np.float32),
        "whhzT": np.ascontiguousarray(Wz.T).astype(np.float32),
        "whhnT": np.ascontiguousarray(Wn.T).astype(np.float32),
        "lcat": lcat,
        "wfcT": np.ascontiguousarray(wfc_half.T).astype(np.float32),
        "bias_bc": bias_bc,
    }


def kernel(**inputs) -> np.ndarray:
    inputs = {k: np.asarray(v, dtype=np.float32) for k, v in inputs.items()}
    nc = _build()
    in_maps = []
    for core in range(NCORES):
        d, g = (0, core) if core < 4 else (1, core - 4)
        in_maps.append(_core_inputs(inputs, d, g))
    res = run_bass_kernel_spmd(nc, in_maps, core_ids=list(range(NCORES)))
    out = np.empty((B, C, O), np.float32)
    for g in range(4):
        o = res.results[g]["out"].reshape(C, BL, O)
        out[g * BL : (g + 1) * BL] = np.transpose(o, (1, 0, 2))
    return out



# revision 5
# speedup vs baseline: 5.0265x; 5.0265x over previous
"""Bidirectional column-chained GRU (vertical BiGRU over image columns) on 8 Trainium2 cores.

Topology: cores 0-3 run the forward GRU chain (batch quarters), cores 4-7 the
backward chain (rows pre-reversed on host). Each core runs the full C*S=16384
sequential GRU steps for its 8 batch rows in feature-major layout (128
partitions = hidden dim, free dim = batch).

Math restructuring (validated vs reference):
  state hp1 = h + 1  (so n-path affine folds shrink the serial chain)
  tanh(x) = 2*sigmoid(2x) - 1  (single ACT table: sigmoid set, no switches)
  The z-gate weights/consts are negated on the host so u = 1-z = sigmoid(+ps_z')
  uses the same scale as r (enables a shared sigmoid table and slice fusion).
  Per column c, for each gate g the rank-1 input contribution
  A_g,t = Wih_g*x_t + const_g is preloaded into PSUM with two K=1 matmuls
  (x row + const ones row; const corrected by -Whh_g@1 for the hp1 shift);
  the recurrent matmul Whh_g @ hp1 then accumulates per step into PSUM slice
  t, so the full pre-activation is read directly from PSUM by ACT/DVE.
  Per step:
    r  = sigmoid(ps_r[t])                 ACT (PSUM src)
    u  = sigmoid(ps_z'[t])  (= 1-z)       ACT
    q  = r * ps_n[t]                      DVE
    w  = q + a_n[t]                       DVE
    v  = sigmoid(2w)                      ACT
    e1 = u * hp1; f = hp1 - e1            DVE
    e2 = 2*u*v                            DVE (scalar_tensor_tensor)
    hp1' = f + e2                         DVE (off matmul path: the recurrent
          matmuls read [f | e2] with a broadcast out-AP so PSUM accumulation
          performs the final add, shortening the serial chain)
  Final per-column features h = hp1 - 1 are collected; the output head
  (fc + relu + softmax) runs on-device with a pairwise AllReduce between the
  fwd/bwd core of each batch group. exp(relu(x)) == max(1, exp(x)).

Transfer plan (axon tunnel costs ~70ms/round trip + ~20ms/MB, so bytes and
dispatches dominate wall time):
  - x ships fp16 without the ones rows (2MB total), upcast to f32 on device.
  - GRU/fc weights, biases and h0 ship once (sharded 1/8 per core) and are
    re-broadcast with an in-kernel AllGather; per-core slices are selected
    with partition-id register DMA offsets at startup.
  - The fc bias broadcast is built on device from the raw 64-float b_fc.
  - Each core writes only its direction's half of the columns, fp16
    (0.25MB total fetched).
  - kernel() keeps a cached jitted shard_map runner: one async dispatch and
    one blocking fetch per call.
"""

import numpy as np

import jax
import jax.numpy as jnp
from jax.sharding import Mesh, PartitionSpec
from jax.experimental.shard_map import shard_map

import concourse.bass as bass
import concourse.bacc as bacc
import concourse.mybir as mybir
import concourse.tile as tile

B, S, C, H, O = 32, 128, 128, 128, 64
NCORES = 8
BL = B // 4          # batch rows per core (4 groups x 2 directions)
SB = S * BL          # rhs columns per image column
HS = SB // 2         # half-column psum width (one bank)
NSTEP = S // 2       # steps per half
f32 = mybir.dt.float32
f16 = mybir.dt.float16
FP = mybir.EngineType

# --- wsh (AllGathered weights blob) layout, in f32 elements ---
_WHH = 3 * H * H                 # one direction's 3 recurrent mats (r, z-neg, n)
_OFF_WHH = 0                     # [2][3][H][H] by dir
_OFF_LCAT = 2 * _WHH             # [2][2][4H] by dir
_OFF_WFC = _OFF_LCAT + 2 * 2 * 4 * H   # [2][H][O] by dir
_OFF_BFC = _OFF_WFC + 2 * H * O        # [O]
_OFF_HP1 = _OFF_BFC + O                # [8][H][BL] by core
_WSH_TOTAL = _OFF_HP1 + NCORES * H * BL
assert _WSH_TOTAL % NCORES == 0
_WSH_SH = _WSH_TOTAL // NCORES


def _emit(nc: bacc.Bacc, n_cols: int = C):
    AF = mybir.ActivationFunctionType
    OPM = mybir.AluOpType.mult

    x_d = nc.dram_tensor("xcols", [C, SB], f16, kind="ExternalInput").ap()
    wsh_d = nc.dram_tensor("wsh", [1, _WSH_SH], f32, kind="ExternalInput").ap()
    out_d = nc.dram_tensor("out", [(C // 2) * BL, O], f16, kind="ExternalOutput").ap()

    wloc_d = nc.dram_tensor("wloc", [1, _WSH_SH], f32, kind="Internal").ap()
    wg_d = nc.dram_tensor("wg", [1, _WSH_TOTAL], f32, kind="Internal").ap()
    wmy_d = nc.dram_tensor("wmy", [1, _WHH + 2 * 4 * H + H * O + O + H * BL], f32,
                           kind="Internal").ap()

    with tile.TileContext(nc) as tc:
        # --- stage the weight shard and AllGather the full blob ---
        nc.sync.dma_start(wloc_d, wsh_d)
        nc.gpsimd.collective_compute(
            "AllGather", mybir.AluOpType.bypass,
            replica_groups=[[0, 1, 2, 3, 4, 5, 6, 7]],
            ins=[wloc_d], outs=[wg_d],
        )
        # --- per-core slice selection (partition-id register offsets) ---
        pid = nc.sync.partition_id()
        d_ = pid // 4
        o = 0
        nc.sync.dma_start(wmy_d[:, o : o + _WHH],
                          wg_d[:, bass.ds(_OFF_WHH + d_ * _WHH, _WHH)])
        o += _WHH
        nc.sync.dma_start(wmy_d[:, o : o + 2 * 4 * H],
                          wg_d[:, bass.ds(_OFF_LCAT + d_ * (2 * 4 * H), 2 * 4 * H)])
        o += 2 * 4 * H
        nc.sync.dma_start(wmy_d[:, o : o + H * O],
                          wg_d[:, bass.ds(_OFF_WFC + d_ * (H * O), H * O)])
        o += H * O
        nc.sync.dma_start(wmy_d[:, o : o + O],
                          wg_d[:, _OFF_BFC : _OFF_BFC + O])
        o += O
        nc.sync.dma_start(wmy_d[:, o : o + H * BL],
                          wg_d[:, bass.ds(_OFF_HP1 + pid * (H * BL), H * BL)])

        with tc.tile_pool(name="const", bufs=1) as cp:
            whhrT = cp.tile([H, H], f32)
            whhzT = cp.tile([H, H], f32)   # negated z weights (host)
            whhnT = cp.tile([H, H], f32)
            lcatW = cp.tile([1, 4 * H], f32)
            lcatC = cp.tile([1, 4 * H], f32)
            wfcT = cp.tile([H, O], f32)
            bfc = cp.tile([1, O], f32)
            hp1 = cp.tile([H, BL], f32)
            ones = cp.tile([1, HS], f32)
            hall = cp.tile([H, C * BL], f32)
            r = cp.tile([H, BL], f32)
            u = cp.tile([H, BL], f32)
            q = cp.tile([H, BL], f32)
            w = cp.tile([H, BL], f32)
            v = cp.tile([H, BL], f32)
            e1 = cp.tile([H, BL], f32)
            fe2 = cp.tile([H, 2 * BL], f32)
            fp_, e2 = fe2[:, 0:BL], fe2[:, BL : 2 * BL]

            o = 0
            nc.sync.dma_start(
                whhrT[:], wmy_d[:, o : o + H * H].rearrange("a (p c) -> (a p) c", p=H))
            nc.sync.dma_start(
                whhzT[:], wmy_d[:, o + H * H : o + 2 * H * H].rearrange(
                    "a (p c) -> (a p) c", p=H))
            nc.sync.dma_start(
                whhnT[:], wmy_d[:, o + 2 * H * H : o + 3 * H * H].rearrange(
                    "a (p c) -> (a p) c", p=H))
            o += _WHH
            nc.sync.dma_start(lcatW[:], wmy_d[:, o : o + 4 * H])
            nc.sync.dma_start(lcatC[:], wmy_d[:, o + 4 * H : o + 2 * 4 * H])
            o += 2 * 4 * H
            nc.sync.dma_start(
                wfcT[:], wmy_d[:, o : o + H * O].rearrange("a (p c) -> (a p) c", p=H))
            o += H * O
            nc.sync.dma_start(bfc[:], wmy_d[:, o : o + O])
            o += O
            nc.sync.dma_start(
                hp1[:], wmy_d[:, o : o + H * BL].rearrange("a (p c) -> (a p) c", p=H))
            nc.sync.dma_start(fp_[:], wmy_d[:, o : o + H * BL].rearrange(
                "a (p c) -> (a p) c", p=H))
            nc.vector.memzero(e2[:])
            nc.vector.memset(ones[:], 1.0)

            with (
                tc.tile_pool(name="col", bufs=2) as colp,
                tc.tile_pool(name="ps", bufs=2, space="PSUM") as psp,
                tc.For_i(
                    0, n_cols, 1,
                    hint_engines=(FP.PE, FP.Activation, FP.DVE),
                ) as cv,
            ):
                xa16 = colp.tile([1, SB], f16, tag="xa16")
                xa = colp.tile([1, SB], f32, tag="xa")
                nc.sync.dma_start(xa16[:], x_d[bass.ds(cv, 1), :])
                nc.vector.tensor_copy(xa[:], xa16[:])

                def preload(half):
                    ps_r = psp.tile([H, HS], f32, tag="ps_r", name=f"ps_r{half}")
                    ps_z = psp.tile([H, HS], f32, tag="ps_z", name=f"ps_z{half}")
                    ps_n = psp.tile([H, HS], f32, tag="ps_n", name=f"ps_n{half}")
                    ps_t = psp.tile([H, HS], f32, tag="ps_t", name=f"ps_t{half}")
                    a_n = colp.tile([H, HS], f32, tag="a_n", name=f"a_n{half}")
                    xh = xa[:, half * HS : (half + 1) * HS]
                    # A_g = Wih_g (x) x_row + const_g (x) ones
                    nc.tensor.matmul(ps_r[:], lcatW[:, 0:H], xh, start=True, stop=False)
                    nc.tensor.matmul(ps_r[:], lcatC[:, 0:H], ones[:], start=False, stop=True)
                    nc.tensor.matmul(ps_z[:], lcatW[:, H : 2 * H], xh, start=True, stop=False)
                    nc.tensor.matmul(ps_z[:], lcatC[:, H : 2 * H], ones[:], start=False, stop=True)
                    # n-gate has no Wih part in the recurrent psum (bhh-only const)
                    nc.tensor.matmul(ps_n[:], lcatC[:, 2 * H : 3 * H], ones[:], start=True, stop=True)
                    nc.tensor.matmul(ps_t[:], lcatW[:, 3 * H : 4 * H], xh, start=True, stop=False)
                    nc.tensor.matmul(ps_t[:], lcatC[:, 3 * H : 4 * H], ones[:], start=False, stop=True)
                    nc.scalar.copy(a_n[:], ps_t[:])
                    return ps_r, ps_z, ps_n, a_n

                def steps(ph, lo, hi):
                    ps_r, ps_z, ps_n, a_n = ph
                    for t in range(lo, hi):
                        sl = slice(t * BL, (t + 1) * BL)
                        hp1v = fe2[:].rearrange("p (a o) -> p a o", a=2)
                        outs = [
                            bass.broadcast_tensor_aps(
                                ps[:, sl].rearrange("p (a o) -> p a o", a=1),
                                hp1v,
                            )[0]
                            for ps in (ps_r, ps_n, ps_z)
                        ]
                        for o_, w_ in zip(outs, (whhrT, whhnT, whhzT)):
                            nc.tensor.matmul(
                                o_, w_[:], hp1v, start=False, stop=True,
                                skip_group_check=True,
                            )
                        nc.scalar.activation(r[:], ps_r[:, sl], AF.Sigmoid)
                        nc.scalar.activation(u[:], ps_z[:, sl], AF.Sigmoid)
                        nc.vector.tensor_mul(q[:], r[:], ps_n[:, sl])
                        nc.vector.tensor_add(w[:], q[:], a_n[:, sl])
                        nc.scalar.activation(v[:], w[:], AF.Sigmoid, scale=2.0)
                        nc.vector.tensor_mul(e1[:], u[:], hp1[:])
                        nc.vector.tensor_sub(fp_[:], hp1[:], e1[:])
                        nc.vector.scalar_tensor_tensor(
                            e2[:], u[:], 2.0, v[:], op0=OPM, op1=OPM
                        )
                        nc.vector.tensor_add(hp1[:], fp_[:], e2[:])

                ph0 = preload(0)
                steps(ph0, 0, 8)
                ph1 = preload(1)
                steps(ph0, 8, NSTEP)
                steps(ph1, 0, NSTEP)
                nc.vector.tensor_scalar_add(
                    hall[:, bass.ts(cv, BL)], hp1[:], -1.0
                )

            # output head: partial logits -> allreduce(fwd,bwd) -> softmax(relu(.))
            # each core writes only its direction's half of the columns.
            with (
                tc.tile_pool(name="fc", bufs=1) as fcp,
                tc.tile_pool(name="psfc", bufs=1, space="PSUM") as psfc,
                tc.tile_pool(name="dramp", bufs=1, space="DRAM") as dp,
            ):
                # fc bias broadcast across partitions: ones_col^T (x) b_fc
                psb = psfc.tile([H, O], f32)
                onesc = fcp.tile([1, H], f32)
                nc.vector.memset(onesc[:], 1.0)
                nc.tensor.matmul(psb[:], onesc[:], bfc[:], start=True, stop=True)
                bias64 = fcp.tile([H, O], f32)
                nc.scalar.copy(bias64[:], psb[:])

                lps = psfc.tile([128, 8 * O], f32)
                for k in range(8):
                    nc.tensor.matmul(
                        lps[:, k * O : (k + 1) * O],
                        hall[:, k * 128 : (k + 1) * 128],
                        wfcT[:],
                        start=True,
                        stop=True,
                    )
                lsb = fcp.tile([128, 8 * O], f32)
                nc.scalar.copy(lsb[:], lps[:])
                lloc = dp.tile([C * BL, O], f32)
                lred = dp.tile([C * BL, O], f32)
                nc.sync.dma_start(
                    lloc.rearrange("(k p) o -> p k o", p=128),
                    lsb[:].rearrange("p (k o) -> p k o", k=8),
                )
                nc.gpsimd.collective_compute(
                    "AllReduce",
                    mybir.AluOpType.add,
                    replica_groups=[[0, 4], [1, 5], [2, 6], [3, 7]],
                    ins=[lloc.opt()],
                    outs=[lred.opt()],
                )
                # fetch only my half of the columns: rows [d*512, d*512+512)
                lsum = fcp.tile([128, 4 * O], f32)
                pid2 = nc.sync.partition_id()
                nc.sync.dma_start(
                    lsum[:].rearrange("p (k o) -> p k o", k=4),
                    lred[bass.ds((pid2 // 4) * ((C // 2) * BL), (C // 2) * BL), :]
                    .rearrange("(k p) o -> p k o", p=128),
                )
                lbi = fcp.tile([128, 4 * O], f32)
                for k in range(4):
                    nc.vector.tensor_add(
                        lbi[:, k * O : (k + 1) * O], lsum[:, k * O : (k + 1) * O],
                        bias64[:],
                    )
                ex = fcp.tile([128, 4 * O], f32)
                nc.scalar.activation(ex[:], lbi[:], AF.Exp)
                # exp(relu(x)) == max(1, exp(x))
                nc.vector.tensor_scalar_max(ex[:], ex[:], 1.0)
                sums = fcp.tile([128, 4], f32)
                nc.vector.tensor_reduce(
                    sums[:],
                    ex[:].rearrange("p (k o) -> p k o", k=4),
                    axis=mybir.AxisListType.X,
                    op=mybir.AluOpType.add,
                )
                rs = fcp.tile([128, 4], f32)
                nc.vector.reciprocal(rs[:], sums[:])
                osb = fcp.tile([128, 4 * O], f16)
                for k in range(4):
                    nc.vector.tensor_scalar_mul(
                        osb[:, k * O : (k + 1) * O],
                        ex[:, k * O : (k + 1) * O],
                        rs[:, k : k + 1],
                    )
                nc.sync.dma_start(
                    out_d.rearrange("(k p) o -> p k o", p=128),
                    osb[:].rearrange("p (k o) -> p k o", k=4),
                )


_CACHE = {}


def _build():
    if "nc" not in _CACHE:
        nc = bacc.Bacc("TRN2", target_bir_lowering=False, debug=False, num_devices=NCORES)
        _emit(nc)
        nc.compile()
        _CACHE["nc"] = nc
    return _CACHE["nc"]


def _pack_x(inputs):
    """fp16 x columns per core: (8*C, SB). Core c<4: group c fwd; c>=4: group
    c-4 with rows (S) reversed."""
    x = inputs["x"]
    out = np.empty((NCORES * C, SB), np.float16)
    for core in range(NCORES):
        d, g = (0, core) if core < 4 else (1, core - 4)
        xg = x[g * BL : (g + 1) * BL]
        if d == 1:
            xg = xg[:, ::-1, :]
        xT = np.transpose(xg, (2, 1, 0)).reshape(C, SB)  # (C, S*BL)
        out[core * C : (core + 1) * C] = xT.astype(np.float16)
    return out


def _pack_w(inputs):
    """f32 weights blob, sharded (8, _WSH_SH)."""
    blob = np.zeros(_WSH_TOTAL, np.float32)
    for d, sfx in ((0, "f"), (1, "b")):
        Wih = inputs[f"Wih_{sfx}"][:, 0]
        Whh = inputs[f"Whh_{sfx}"]
        bih = inputs[f"bih_{sfx}"]
        bhh = inputs[f"bhh_{sfx}"]
        Wr, Wz, Wn = Whh[:H], Whh[H : 2 * H], Whh[2 * H :]
        # transposed recurrent mats; z-gate negated so u = sigmoid(+ps_z')
        whh = np.stack([Wr.T, -Wz.T, Wn.T]).astype(np.float32)
        blob[_OFF_WHH + d * _WHH : _OFF_WHH + (d + 1) * _WHH] = whh.ravel()
        lcat = np.zeros((2, 4 * H), np.float32)
        lcat[0, 0:H] = Wih[:H]
        lcat[1, 0:H] = bih[:H] + bhh[:H] - Wr.sum(1)
        lcat[0, H : 2 * H] = -Wih[H : 2 * H]
        lcat[1, H : 2 * H] = -(bih[H : 2 * H] + bhh[H : 2 * H] - Wz.sum(1))
        lcat[1, 2 * H : 3 * H] = bhh[2 * H :] - Wn.sum(1)
        lcat[0, 3 * H : 4 * H] = Wih[2 * H :]
        lcat[1, 3 * H : 4 * H] = bih[2 * H :]
        blob[_OFF_LCAT + d * 8 * H : _OFF_LCAT + (d + 1) * 8 * H] = lcat.ravel()
        wfc_half = inputs["W_fc"][:, :H] if d == 0 else inputs["W_fc"][:, H:]
        blob[_OFF_WFC + d * H * O : _OFF_WFC + (d + 1) * H * O] = (
            np.ascontiguousarray(wfc_half.T).astype(np.float32).ravel()
        )
    blob[_OFF_BFC : _OFF_BFC + O] = inputs["b_fc"].astype(np.float32)
    for core in range(NCORES):
        d, g = (0, core) if core < 4 else (1, core - 4)
        hp10 = (inputs["h_prev"][d, g * BL : (g + 1) * BL] + 1.0).T.astype(np.float32)
        blob[_OFF_HP1 + core * H * BL : _OFF_HP1 + (core + 1) * H * BL] = hp10.ravel()
    return blob.reshape(NCORES, _WSH_SH)


def _make_runner(nc):
    """Cached jitted shard_map runner over 8 cores (axon bass_exec path)."""
    from concourse.bass2jax import (
        _bass_exec_p,
        partition_id_tensor,
        install_neuronx_cc_hook,
    )

    install_neuronx_cc_hook()
    partition_name = nc.partition_id_tensor.name if nc.partition_id_tensor else None
    in_names, out_names, out_avals, zero_shapes = [], [], [], []
    for alloc in nc.m.functions[0].allocations:
        if not isinstance(alloc, mybir.MemoryLocationSet):
            continue
        name = alloc.memorylocations[0].name
        if alloc.kind == "ExternalInput":
            if name != partition_name:
                in_names.append(name)
        elif alloc.kind == "ExternalOutput":
            shape = tuple(alloc.tensor_shape)
            dtype = mybir.dt.np(alloc.dtype)
            out_names.append(name)
            out_avals.append(jax.core.ShapedArray(shape, dtype))
            zero_shapes.append((shape, dtype))
    n_params = len(in_names)
    n_outs = len(out_avals)
    in_names_all = in_names + out_names + ([partition_name] if partition_name else [])
    donate = tuple(range(n_params, n_params + n_outs))

    def _body(*args):
        operands = list(args)
        if partition_name is not None:
            operands.append(partition_id_tensor())
        outs = _bass_exec_p.bind(
            *operands,
            out_avals=tuple(out_avals),
            in_names=tuple(in_names_all),
            out_names=tuple(out_names),
            lowering_input_output_aliases=(),
            sim_require_finite=True,
            sim_require_nnan=True,
            nc=nc,
        )
        return tuple(outs)

    devices = jax.devices()[:NCORES]
    mesh = Mesh(np.asarray(devices), ("core",))
    in_specs = (PartitionSpec("core"),) * (n_params + n_outs)
    out_specs = (PartitionSpec("core"),) * len(out_names)
    sharded = jax.jit(
        shard_map(_body, mesh=mesh, in_specs=in_specs, out_specs=out_specs,
                  check_rep=False),
        donate_argnums=donate,
        keep_unused=True,
    )

    def run(in_map_concat):
        args = [in_map_concat[name] for name in in_names]
        zeros = [
            np.zeros((NCORES * s[0], *s[1:]), dt) for s, dt in zero_shapes
        ]
        out_arrs = sharded(*args, *zeros)
        return {
            name: np.asarray(out_arrs[i]) for i, name in enumerate(out_names)
        }

    return run


def kernel(**inputs) -> np.ndarray:
    inputs = {k: np.asarray(v, dtype=np.float32) for k, v in inputs.items()}
    nc = _build()
    if "runner" not in _CACHE:
        _CACHE["runner"] = _make_runner(nc)
    run = _CACHE["runner"]
    res = run({"xcols": _pack_x(inputs), "wsh": _pack_w(inputs)})
    o16 = res["out"].reshape(NCORES, C // 2, BL, O).astype(np.float32)
    out = np.empty((B, C, O), np.float32)
    for g in range(4):
        out[g * BL : (g + 1) * BL, : C // 2] = np.transpose(o16[g], (1, 0, 2))
        out[g * BL : (g + 1) * BL, C // 2 :] = np.transpose(o16[g + 4], (1, 0, 2))
    return out


# revision 11
# speedup vs baseline: 7.7340x; 1.5386x over previous
"""Bidirectional column-chained GRU (vertical BiGRU over image columns) on 8 Trainium2 cores.

Topology: cores 0-3 run the forward GRU chain (batch quarters), cores 4-7 the
backward chain (rows pre-reversed on host). Each core runs the full C*S=16384
sequential GRU steps for its 8 batch rows in feature-major layout (128
partitions = hidden dim, free dim = batch).

Math restructuring (validated vs reference):
  state hp1 = h + 1  (so n-path affine folds shrink the serial chain)
  tanh(x) = 2*sigmoid(2x) - 1  (single ACT table: sigmoid set, no switches)
  The z-gate weights/consts are negated on the host so u = 1-z = sigmoid(+ps_z')
  uses the same scale as r (enables a shared sigmoid table and slice fusion).
  Per column c, for each gate g the rank-1 input contribution
  A_g,t = Wih_g*x_t + const_g is preloaded into PSUM with two K=1 matmuls
  (x row + const ones row; const corrected by -Whh_g@1 for the hp1 shift);
  the recurrent matmul Whh_g @ hp1 then accumulates per step into PSUM slice
  t, so the full pre-activation is read directly from PSUM by ACT/DVE.
  Per step:
    r  = sigmoid(ps_r[t])                 ACT (PSUM src)
    u  = sigmoid(ps_z'[t])  (= 1-z)       ACT
    q  = r * ps_n[t]                      DVE
    w  = q + a_n[t]                       DVE
    v  = sigmoid(2w)                      ACT
    e1 = u * hp1; f = hp1 - e1            DVE
    e2 = 2*u*v                            DVE (scalar_tensor_tensor)
    hp1' = f + e2                         DVE (off matmul path: the recurrent
          matmuls read [f | e2] with a broadcast out-AP so PSUM accumulation
          performs the final add, shortening the serial chain)
  Final per-column features h = hp1 - 1 are collected; the output head
  (fc + relu + softmax) runs on-device with a pairwise AllReduce between the
  fwd/bwd core of each batch group. exp(relu(x)) == max(1, exp(x)).

Transfer plan (axon tunnel costs ~70ms/round trip + ~20ms/MB, so bytes and
dispatches dominate wall time):
  - x ships fp16 without the ones rows (2MB total), upcast to f32 on device.
  - GRU/fc weights, biases and h0 ship once (sharded 1/8 per core) and are
    re-broadcast with an in-kernel AllGather; per-core slices are selected
    with partition-id register DMA offsets at startup.
  - The fc bias broadcast is built on device from the raw 64-float b_fc.
  - Each core writes only its direction's half of the columns, fp16
    (0.25MB total fetched).
  - kernel() keeps a cached jitted shard_map runner: one async dispatch and
    one blocking fetch per call.
"""

import numpy as np

import jax
import jax.numpy as jnp
from jax.sharding import Mesh, PartitionSpec
from jax.experimental.shard_map import shard_map

import concourse.bass as bass
import concourse.bacc as bacc
import concourse.mybir as mybir
import concourse.tile as tile

B, S, C, H, O = 32, 128, 128, 128, 64
NCORES = 8
BL = B // 4          # batch rows per core (4 groups x 2 directions)
SB = S * BL          # rhs columns per image column
HS = SB // 2         # half-column psum width (one bank)
NSTEP = S // 2       # steps per half
f32 = mybir.dt.float32
f16 = mybir.dt.float16
FP = mybir.EngineType

# --- wsh (AllGathered weights blob) layout, in f32 elements ---
_WHH = 3 * H * H // 2            # one dir's 3 recurrent mats (r, z-neg, n), fp16-packed
_OFF_WHH = 0                     # [2][3][H][H] by dir (fp16 pairs in f32 slots)
_OFF_LCAT = 2 * _WHH             # [2][2][4H] by dir
_OFF_WFC = _OFF_LCAT + 2 * 2 * 4 * H   # [2][H][O] by dir
_OFF_BFC = _OFF_WFC + 2 * H * O        # [O]
_OFF_HP1 = _OFF_BFC + O                # [8][H][BL] by core
_WSH_TOTAL = _OFF_HP1 + NCORES * H * BL
assert _WSH_TOTAL % NCORES == 0
_WSH_SH = _WSH_TOTAL // NCORES


def _emit(nc: bacc.Bacc, n_cols: int = C):
    AF = mybir.ActivationFunctionType
    OPM = mybir.AluOpType.mult

    x_d = nc.dram_tensor("xcols", [C, SB], f16, kind="ExternalInput").ap()
    wsh_d = nc.dram_tensor("wsh", [1, _WSH_SH], f32, kind="ExternalInput").ap()
    out_d = nc.dram_tensor("out", [(C // 2) * BL, O], f16, kind="ExternalOutput").ap()

    wloc_d = nc.dram_tensor("wloc", [1, _WSH_SH], f32, kind="Internal").ap()
    wg_d = nc.dram_tensor("wg", [1, _WSH_TOTAL], f32, kind="Internal").ap()
    wmy_d = nc.dram_tensor("wmy", [1, _WHH + 2 * 4 * H + H * O + O + H * BL], f32,
                           kind="Internal").ap()

    with tile.TileContext(nc) as tc:
        # --- stage the weight shard and AllGather the full blob ---
        nc.sync.dma_start(wloc_d, wsh_d)
        nc.gpsimd.collective_compute(
            "AllGather", mybir.AluOpType.bypass,
            replica_groups=[[0, 1, 2, 3, 4, 5, 6, 7]],
            ins=[wloc_d], outs=[wg_d],
        )
        # --- per-core slice selection (partition-id register offsets) ---
        pid = nc.sync.partition_id()
        d_ = pid // 4
        o = 0
        nc.sync.dma_start(wmy_d[:, o : o + _WHH],
                          wg_d[:, bass.ds(_OFF_WHH + d_ * _WHH, _WHH)])
        o += _WHH
        nc.sync.dma_start(wmy_d[:, o : o + 2 * 4 * H],
                          wg_d[:, bass.ds(_OFF_LCAT + d_ * (2 * 4 * H), 2 * 4 * H)])
        o += 2 * 4 * H
        nc.sync.dma_start(wmy_d[:, o : o + H * O],
                          wg_d[:, bass.ds(_OFF_WFC + d_ * (H * O), H * O)])
        o += H * O
        nc.sync.dma_start(wmy_d[:, o : o + O],
                          wg_d[:, _OFF_BFC : _OFF_BFC + O])
        o += O
        nc.sync.dma_start(wmy_d[:, o : o + H * BL],
                          wg_d[:, bass.ds(_OFF_HP1 + pid * (H * BL), H * BL)])

        with tc.tile_pool(name="const", bufs=1) as cp:
            whhrT = cp.tile([H, H], f16)
            whhzT = cp.tile([H, H], f16)   # negated z weights (host)
            whhnT = cp.tile([H, H], f16)
            lcatW = cp.tile([1, 4 * H], f32)
            lcatC = cp.tile([1, 4 * H], f32)
            wfcT = cp.tile([H, O], f32)
            bfc = cp.tile([1, O], f32)
            hp1 = cp.tile([H, BL], f32)
            ones = cp.tile([1, HS], f32)
            hall = cp.tile([H, C * BL], f32)
            ru = cp.tile([H, 2 * BL], f32)
            r, u = ru[:, 0:BL], ru[:, BL : 2 * BL]
            q = cp.tile([H, BL], f32)
            w = cp.tile([H, BL], f32)
            v = cp.tile([H, BL], f32)
            e1 = cp.tile([H, BL], f32)
            fe2 = cp.tile([H, 2 * BL], f16)
            fp_, e2 = fe2[:, 0:BL], fe2[:, BL : 2 * BL]

            o = 0
            hh = H * H // 2  # f32 slots per fp16 matrix
            wmy16 = wmy_d[:, 0:_WHH].bitcast(f16)  # [1, 3*H*H] f16
            nc.sync.dma_start(
                whhrT[:], wmy16[:, 0 : H * H].rearrange("a (p c) -> (a p) c", p=H))
            nc.sync.dma_start(
                whhzT[:], wmy16[:, H * H : 2 * H * H].rearrange(
                    "a (p c) -> (a p) c", p=H))
            nc.sync.dma_start(
                whhnT[:], wmy16[:, 2 * H * H : 3 * H * H].rearrange(
                    "a (p c) -> (a p) c", p=H))
            o += _WHH
            nc.sync.dma_start(lcatW[:], wmy_d[:, o : o + 4 * H])
            nc.sync.dma_start(lcatC[:], wmy_d[:, o + 4 * H : o + 2 * 4 * H])
            o += 2 * 4 * H
            nc.sync.dma_start(
                wfcT[:], wmy_d[:, o : o + H * O].rearrange("a (p c) -> (a p) c", p=H))
            o += H * O
            nc.sync.dma_start(bfc[:], wmy_d[:, o : o + O])
            o += O
            nc.sync.dma_start(
                hp1[:], wmy_d[:, o : o + H * BL].rearrange("a (p c) -> (a p) c", p=H))
            nc.vector.memzero(e2[:])
            nc.vector.tensor_copy(fp_[:], hp1[:])
            nc.vector.memset(ones[:], 1.0)

            with (
                tc.tile_pool(name="col", bufs=2) as colp,
                tc.tile_pool(name="ps", bufs=2, space="PSUM") as psp,
                tc.For_i(
                    0, n_cols, 1,
                    hint_engines=(FP.PE, FP.Activation, FP.DVE),
                ) as cv,
            ):
                xa16 = colp.tile([1, SB], f16, tag="xa16")
                xa = colp.tile([1, SB], f32, tag="xa")
                nc.sync.dma_start(xa16[:], x_d[bass.ds(cv, 1), :])
                nc.vector.tensor_copy(xa[:], xa16[:])

                def preload(half):
                    ps_rz = psp.tile([H, 2 * HS], f32, tag="ps_rz", name=f"ps_rz{half}")
                    ps_n = psp.tile([H, HS], f32, tag="ps_n", name=f"ps_n{half}")
                    ps_t = psp.tile([H, HS], f32, tag="ps_t", name=f"ps_t{half}")
                    a_n = colp.tile([H, HS], f32, tag="a_n", name=f"a_n{half}")
                    xh = xa[:, half * HS : (half + 1) * HS]
                    # A_g = Wih_g (x) x_row + const_g (x) ones
                    nc.tensor.matmul(ps_rz[:, 0:HS], lcatW[:, 0:H], xh, start=True, stop=False)
                    nc.tensor.matmul(ps_rz[:, 0:HS], lcatC[:, 0:H], ones[:], start=False, stop=True)
                    nc.tensor.matmul(ps_rz[:, HS : 2 * HS], lcatW[:, H : 2 * H], xh, start=True, stop=False)
                    nc.tensor.matmul(ps_rz[:, HS : 2 * HS], lcatC[:, H : 2 * H], ones[:], start=False, stop=True)
                    # n-gate has no Wih part in the recurrent psum (bhh-only const)
                    nc.tensor.matmul(ps_n[:], lcatC[:, 2 * H : 3 * H], ones[:], start=True, stop=True)
                    nc.tensor.matmul(ps_t[:], lcatW[:, 3 * H : 4 * H], xh, start=True, stop=False)
                    nc.tensor.matmul(ps_t[:], lcatC[:, 3 * H : 4 * H], ones[:], start=False, stop=True)
                    nc.scalar.copy(a_n[:], ps_t[:])
                    return ps_rz, ps_n, a_n

                def steps(ph, lo, hi):
                    ps_rz, ps_n, a_n = ph
                    ps_rz3 = ps_rz[:].rearrange("p (a o) -> p a o", a=2)
                    ru3 = ru[:].rearrange("p (a o) -> p a o", a=2)
                    for t in range(lo, hi):
                        sl = slice(t * BL, (t + 1) * BL)
                        hp1v = fe2[:].rearrange("p (a o) -> p a o", a=2)
                        outs = [
                            bass.broadcast_tensor_aps(
                                ps[:, sl].rearrange("p (a o) -> p a o", a=1),
                                hp1v,
                            )[0]
                            for ps in (ps_rz[:, 0:HS], ps_rz[:, HS : 2 * HS], ps_n)
                        ]
                        for o_, w_ in zip(outs, (whhrT, whhzT, whhnT)):
                            nc.tensor.matmul(
                                o_, w_[:], hp1v, start=False, stop=True,
                                skip_group_check=True,
                            )
                        # one ACT reads both gate slices (r | u) via strided AP
                        nc.scalar.activation(
                            ru3, ps_rz3[:, :, sl], AF.Sigmoid
                        )
                        nc.vector.tensor_mul(q[:], r[:], ps_n[:, sl])
                        nc.vector.tensor_add(w[:], q[:], a_n[:, sl])
                        nc.scalar.activation(v[:], w[:], AF.Sigmoid, scale=2.0)
                        nc.vector.tensor_mul(e1[:], u[:], hp1[:])
                        nc.vector.tensor_sub(fp_[:], hp1[:], e1[:])
                        nc.vector.scalar_tensor_tensor(
                            e2[:], u[:], 2.0, v[:], op0=OPM, op1=OPM
                        )
                        nc.vector.tensor_add(hp1[:], fp_[:], e2[:])

                ph0 = preload(0)
                steps(ph0, 0, 8)
                ph1 = preload(1)
                steps(ph0, 8, NSTEP)
                steps(ph1, 0, NSTEP)
                nc.vector.tensor_scalar_add(
                    hall[:, bass.ts(cv, BL)], hp1[:], -1.0
                )

            # output head: partial logits -> allreduce(fwd,bwd) -> softmax(relu(.))
            # each core writes only its direction's half of the columns.
            with (
                tc.tile_pool(name="fc", bufs=1) as fcp,
                tc.tile_pool(name="psfc", bufs=1, space="PSUM") as psfc,
                tc.tile_pool(name="dramp", bufs=1, space="DRAM") as dp,
            ):
                # fc bias broadcast across partitions: ones_col^T (x) b_fc
                psb = psfc.tile([H, O], f32)
                onesc = fcp.tile([1, H], f32)
                nc.vector.memset(onesc[:], 1.0)
                nc.tensor.matmul(psb[:], onesc[:], bfc[:], start=True, stop=True)
                bias64 = fcp.tile([H, O], f32)
                nc.scalar.copy(bias64[:], psb[:])

                lps = psfc.tile([128, 8 * O], f32)
                for k in range(8):
                    nc.tensor.matmul(
                        lps[:, k * O : (k + 1) * O],
                        hall[:, k * 128 : (k + 1) * 128],
                        wfcT[:],
                        start=True,
                        stop=True,
                    )
                lsb = fcp.tile([128, 8 * O], f32)
                nc.scalar.copy(lsb[:], lps[:])
                lloc = dp.tile([C * BL, O], f32)
                lred = dp.tile([C * BL, O], f32)
                nc.sync.dma_start(
                    lloc.rearrange("(k p) o -> p k o", p=128),
                    lsb[:].rearrange("p (k o) -> p k o", k=8),
                )
                nc.gpsimd.collective_compute(
                    "AllReduce",
                    mybir.AluOpType.add,
                    replica_groups=[[0, 4], [1, 5], [2, 6], [3, 7]],
                    ins=[lloc.opt()],
                    outs=[lred.opt()],
                )
                # fetch only my half of the columns: rows [d*512, d*512+512)
                lsum = fcp.tile([128, 4 * O], f32)
                pid2 = nc.sync.partition_id()
                nc.sync.dma_start(
                    lsum[:].rearrange("p (k o) -> p k o", k=4),
                    lred[bass.ds((pid2 // 4) * ((C // 2) * BL), (C // 2) * BL), :]
                    .rearrange("(k p) o -> p k o", p=128),
                )
                lbi = fcp.tile([128, 4 * O], f32)
                for k in range(4):
                    nc.vector.tensor_add(
                        lbi[:, k * O : (k + 1) * O], lsum[:, k * O : (k + 1) * O],
                        bias64[:],
                    )
                ex = fcp.tile([128, 4 * O], f32)
                nc.scalar.activation(ex[:], lbi[:], AF.Exp)
                # exp(relu(x)) == max(1, exp(x))
                nc.vector.tensor_scalar_max(ex[:], ex[:], 1.0)
                sums = fcp.tile([128, 4], f32)
                nc.vector.tensor_reduce(
                    sums[:],
                    ex[:].rearrange("p (k o) -> p k o", k=4),
                    axis=mybir.AxisListType.X,
                    op=mybir.AluOpType.add,
                )
                rs = fcp.tile([128, 4], f32)
                nc.vector.reciprocal(rs[:], sums[:])
                osb = fcp.tile([128, 4 * O], f16)
                for k in range(4):
                    nc.vector.tensor_scalar_mul(
                        osb[:, k * O : (k + 1) * O],
                        ex[:, k * O : (k + 1) * O],
                        rs[:, k : k + 1],
                    )
                nc.sync.dma_start(
                    out_d.rearrange("(k p) o -> p k o", p=128),
                    osb[:].rearrange("p (k o) -> p k o", k=4),
                )


_CACHE = {}


def _build():
    if "nc" not in _CACHE:
        nc = bacc.Bacc("TRN2", target_bir_lowering=False, debug=False, num_devices=NCORES)
        _emit(nc)
        nc.compile()
        _CACHE["nc"] = nc
    return _CACHE["nc"]


def _pack_x(inputs):
    """fp16 x columns per core: (8*C, SB). Core c<4: group c fwd; c>=4: group
    c-4 with rows (S) reversed."""
    x = inputs["x"]
    out = np.empty((NCORES * C, SB), np.float16)
    for core in range(NCORES):
        d, g = (0, core) if core < 4 else (1, core - 4)
        xg = x[g * BL : (g + 1) * BL]
        if d == 1:
            xg = xg[:, ::-1, :]
        xT = np.transpose(xg, (2, 1, 0)).reshape(C, SB)  # (C, S*BL)
        out[core * C : (core + 1) * C] = xT.astype(np.float16)
    return out


def _pack_w(inputs):
    """f32 weights blob, sharded (8, _WSH_SH)."""
    blob = np.zeros(_WSH_TOTAL, np.float32)
    for d, sfx in ((0, "f"), (1, "b")):
        Wih = inputs[f"Wih_{sfx}"][:, 0]
        Whh = inputs[f"Whh_{sfx}"]
        bih = inputs[f"bih_{sfx}"]
        bhh = inputs[f"bhh_{sfx}"]
        Wr, Wz, Wn = Whh[:H], Whh[H : 2 * H], Whh[2 * H :]
        # transposed recurrent mats; z-gate negated so u = sigmoid(+ps_z');
        # fp16, packed pairwise into f32 blob slots
        whh = np.stack([Wr.T, -Wz.T, Wn.T]).astype(np.float16)
        blob[_OFF_WHH + d * _WHH : _OFF_WHH + (d + 1) * _WHH] = (
            whh.ravel().view(np.float32)
        )
        lcat = np.zeros((2, 4 * H), np.float32)
        lcat[0, 0:H] = Wih[:H]
        lcat[1, 0:H] = bih[:H] + bhh[:H] - Wr.sum(1)
        lcat[0, H : 2 * H] = -Wih[H : 2 * H]
        lcat[1, H : 2 * H] = -(bih[H : 2 * H] + bhh[H : 2 * H] - Wz.sum(1))
        lcat[1, 2 * H : 3 * H] = bhh[2 * H :] - Wn.sum(1)
        lcat[0, 3 * H : 4 * H] = Wih[2 * H :]
        lcat[1, 3 * H : 4 * H] = bih[2 * H :]
        blob[_OFF_LCAT + d * 8 * H : _OFF_LCAT + (d + 1) * 8 * H] = lcat.ravel()
        wfc_half = inputs["W_fc"][:, :H] if d == 0 else inputs["W_fc"][:, H:]
        blob[_OFF_WFC + d * H * O : _OFF_WFC + (d + 1) * H * O] = (
            np.ascontiguousarray(wfc_half.T).astype(np.float32).ravel()
        )
    blob[_OFF_BFC : _OFF_BFC + O] = inputs["b_fc"].astype(np.float32)
    for core in range(NCORES):
        d, g = (0, core) if core < 4 else (1, core - 4)
        hp10 = (inputs["h_prev"][d, g * BL : (g + 1) * BL] + 1.0).T.astype(np.float32)
        blob[_OFF_HP1 + core * H * BL : _OFF_HP1 + (core + 1) * H * BL] = hp10.ravel()
    return blob.reshape(NCORES, _WSH_SH)


def _make_runner(nc):
    """Cached jitted shard_map runner over 8 cores (axon bass_exec path)."""
    from concourse.bass2jax import (
        _bass_exec_p,
        partition_id_tensor,
        install_neuronx_cc_hook,
    )

    install_neuronx_cc_hook()
    partition_name = nc.partition_id_tensor.name if nc.partition_id_tensor else None
    in_names, out_names, out_avals, zero_shapes = [], [], [], []
    for alloc in nc.m.functions[0].allocations:
        if not isinstance(alloc, mybir.MemoryLocationSet):
            continue
        name = alloc.memorylocations[0].name
        if alloc.kind == "ExternalInput":
            if name != partition_name:
                in_names.append(name)
        elif alloc.kind == "ExternalOutput":
            shape = tuple(alloc.tensor_shape)
            dtype = mybir.dt.np(alloc.dtype)
            out_names.append(name)
            out_avals.append(jax.core.ShapedArray(shape, dtype))
            zero_shapes.append((shape, dtype))
    n_params = len(in_names)
    n_outs = len(out_avals)
    in_names_all = in_names + out_names + ([partition_name] if partition_name else [])
    donate = tuple(range(n_params, n_params + n_outs))

    def _body(*args):
        operands = list(args)
        if partition_name is not None:
            operands.append(partition_id_tensor())
        outs = _bass_exec_p.bind(
            *operands,
            out_avals=tuple(out_avals),
            in_names=tuple(in_names_all),
            out_names=tuple(out_names),
            lowering_input_output_aliases=(),
            sim_require_finite=True,
            sim_require_nnan=True,
            nc=nc,
        )
        return tuple(outs)

    devices = jax.devices()[:NCORES]
    mesh = Mesh(np.asarray(devices), ("core",))
    in_specs = (PartitionSpec("core"),) * (n_params + n_outs)
    out_specs = (PartitionSpec("core"),) * len(out_names)
    sharded = jax.jit(
        shard_map(_body, mesh=mesh, in_specs=in_specs, out_specs=out_specs,
                  check_rep=False),
        donate_argnums=donate,
        keep_unused=True,
    )

    def run(in_map_concat):
        args = [in_map_concat[name] for name in in_names]
        zeros = [
            np.zeros((NCORES * s[0], *s[1:]), dt) for s, dt in zero_shapes
        ]
        out_arrs = sharded(*args, *zeros)
        return {
            name: np.asarray(out_arrs[i]) for i, name in enumerate(out_names)
        }

    return run


def kernel(**inputs) -> np.ndarray:
    inputs = {k: np.asarray(v, dtype=np.float32) for k, v in inputs.items()}
    nc = _build()
    if "runner" not in _CACHE:
        _CACHE["runner"] = _make_runner(nc)
    run = _CACHE["runner"]
    res = run({"xcols": _pack_x(inputs), "wsh": _pack_w(inputs)})
    o16 = res["out"].reshape(NCORES, C // 2, BL, O).astype(np.float32)
    out = np.empty((B, C, O), np.float32)
    for g in range(4):
        out[g * BL : (g + 1) * BL, : C // 2] = np.transpose(o16[g], (1, 0, 2))
        out[g * BL : (g + 1) * BL, C // 2 :] = np.transpose(o16[g + 4], (1, 0, 2))
    return out
